# revision 45
# baseline (speedup 1.0000x reference)
"""DeepGCN (gnn_message_passing) Trainium2 Bass kernel, 8-way node-sharded SPMD.

Strategy (per core, nodes sharded 8 ways):
- Activations kept transposed hT [128 feats, RPAD rows] in SBUF.
- Dense y = h@W + b: PE matmuls lhsT=hT-tile rhs=W (+rank-1 ones-matmul bias)
  -> row-major y tiles -> DMA to DRAM [HALF, 2F] (two column halves) ->
  ONE AllGather per layer -> table [TBL=HALF*8, 2F] fp16 (Shared).  Splitting
  each rank's rows into two column-halves keeps every gather index < 32768
  (int16).  All four layers use F=128 fp16 tables (w2/b2 zero-padded 64->128)
  so one selector array serves every layer.
- spmm out[r] = sum_e val[e] * y[col[e]]: dma_gather 128 edges/tile into
  partitions (4 SWDGE queues round-robin: queue q's descriptor prep runs on
  Q7 core pair (2q, 2q+1), so 4 queues spread prep over all 8 Q7 cores),
  host-precomputed selector SEL[e,r] = (rowrel[e]==r)*val[e] streamed from
  DRAM (identical for all layers), PE matmul g.T @ SEL accumulated in PSUM
  per 128-row block -> transposed result updates hT (relu/residual fused).
- Final spmm uses lhsT=SEL, rhs=g -> row-major [rows, 128] -> first 64 cols
  are the output shard.

Edges are preprocessed on host (numpy): sorted by destination row-block, split
per block into the two table halves, padded to a fixed number of 128-edge
tiles per (block, half) so one static program serves all 8 cores.
"""

import numpy as np

import concourse.bacc as bacc
import concourse.bass as bass
import concourse.mybir as mybir
import concourse.tile as tile
from concourse import library_config
from concourse.bass_utils import run_bass_kernel_spmd

NCORES = 8
P = 128


class Cfg:
    def __init__(self, N=40000, E=640000, DIN=256, H=128, C=64, L=2, SBB=5):
        assert N % NCORES == 0
        self.N, self.E, self.DIN, self.H, self.C, self.L = N, E, DIN, H, C, L
        self.NSH = N // NCORES                    # rows per core
        self.NBLK = -(-self.NSH // P)             # 128-row blocks per core
        self.RPAD = self.NBLK * P
        assert self.NSH % 2 == 0
        self.HALF = self.NSH // 2                 # rows per table half per core
        self.TBL = self.HALF * NCORES             # rows per gather table
        assert self.TBL < 32768, "gather indices must fit int16"
        assert self.NBLK % SBB == 0
        self.SBB = SBB                            # blocks per superblock
        self.NSB = self.NBLK // SBB
        self.tdt = mybir.dt.float16
        self.tnp = np.float16


CFG_FULL = Cfg()


# ---------------------------------------------------------------- host side


def _pack_idx(idx_flat):
    """[n] int16 -> [128, n//16]: slot i -> partition i%16, col i//16, x8 replicated."""
    n = idx_flat.shape[-1]
    t = idx_flat.reshape(*idx_flat.shape[:-1], n // 16, 16)
    t = np.swapaxes(t, -1, -2)                    # [..., 16, n//16]
    return np.tile(t, (1,) * (t.ndim - 2) + (8, 1)).astype(np.int16)


def preprocess(cfg, x, edge_row, edge_col, edge_val):
    """Shard x, build per-core gather/selector metadata. Returns (per_core, TPB)."""
    er = np.asarray(edge_row).astype(np.int64)
    ec = np.asarray(edge_col).astype(np.int64)
    ev = np.asarray(edge_val).astype(np.float32)
    x = np.asarray(x, np.float32)

    owner = er // cfg.NSH
    row_loc = er % cfg.NSH
    blk = row_loc // P                            # block within core
    rel = (row_loc % P).astype(np.int64)
    c_owner = ec // cfg.NSH
    c_loc = ec % cfg.NSH
    half = (c_loc >= cfg.HALF).astype(np.int64)
    tbl_idx = (c_owner * cfg.HALF + c_loc - half * cfg.HALF).astype(np.int64)

    cores = []
    max_cnt = 0
    for r in range(NCORES):
        m = owner == r
        cores.append((blk[m], half[m], tbl_idx[m], ev[m], rel[m]))
        key = blk[m] * 2 + half[m]
        cnt = np.bincount(key, minlength=cfg.NBLK * 2)
        max_cnt = max(max_cnt, int(cnt.max()))
    TPB = max(1, -(-max_cnt // P))                # tiles per (block, half)
    NIDX = cfg.SBB * TPB * P                      # gather-call size
    NPT = cfg.SBB * TPB

    per_core = []
    for r in range(NCORES):
        b, h, ti, v, rl = cores[r]
        key = b * 2 + h
        # secondary sort by table index: monotone gather addresses within each
        # (block, half) group give far better HBM row locality
        order = np.argsort(key * 32768 + ti, kind="stable")
        b, h, ti, v, rl = b[order], h[order], ti[order], v[order], rl[order]
        cnt = np.bincount(key[order], minlength=cfg.NBLK * 2)
        # slot of edge j within its (b,h) group
        within = np.arange(len(b)) - np.repeat(
            np.concatenate([[0], np.cumsum(cnt)[:-1]]), cnt)
        # flat slot in [h, s, NIDX] layout
        s = b // cfg.SBB
        bb = b % cfg.SBB
        slot = bb * TPB * P + within
        idx_arr = np.zeros((2, cfg.NSB, NIDX), np.int16)
        idx_arr[h, s, slot] = ti.astype(np.int16)

        # host-built selector: SEL[h, s, e%128, (e//128)*128 + rowrel] = val
        sel = np.zeros((2, cfg.NSB, NIDX, P), np.float16)
        sel[h, s, slot, rl] = v.astype(np.float16)
        # [2, NSB, NIDX, P] -> [2, NSB, NPT, 128e, 128r] -> [2, NSB, 128e, NPT*128r]
        sel = sel.reshape(2, cfg.NSB, NPT, P, P).transpose(0, 1, 3, 2, 4)
        sel = np.ascontiguousarray(sel.reshape(2, cfg.NSB, P, NPT * P))

        xT = np.zeros((cfg.DIN, cfg.RPAD), np.float16)
        xT[:, : cfg.NSH] = x[r * cfg.NSH:(r + 1) * cfg.NSH].T.astype(np.float16)
        per_core.append(dict(
            xT=np.ascontiguousarray(xT),
            idx=_pack_idx(idx_arr),                       # [2,NSB,128,NIDX//16]
            sel=sel,                                      # [2,NSB,128,NPT*128]
        ))
    return per_core, TPB


# -------------------------------------------------------------- device side


def build_program(cfg, TPB, dt_val, no_cc=False):
    H, DIN, L, C = cfg.H, cfg.DIN, cfg.L, cfg.C
    NIDX = cfg.SBB * TPB * P
    NPT = cfg.SBB * TPB
    F = H                                         # uniform table feature dim
    NLAY = L + 2

    nc = bacc.Bacc("TRN2", target_bir_lowering=False, debug=False,
                   num_devices=NCORES, num_swdge_queues=4,
                   dynamic_dma_scratch_size=32768)
    f32 = mybir.dt.float32
    f16 = cfg.tdt

    xT_d = nc.dram_tensor("xT", [DIN, cfg.RPAD], f16, kind="ExternalInput")
    w1_d = nc.dram_tensor("w1", [DIN, H], f16, kind="ExternalInput")
    b1_d = nc.dram_tensor("b1", [1, H], f16, kind="ExternalInput")
    wm_d = nc.dram_tensor("wm", [L, H, H], f32, kind="ExternalInput")
    bm_d = nc.dram_tensor("bm", [L, 1, H], f32, kind="ExternalInput")
    w2_d = nc.dram_tensor("w2", [H, H], f32, kind="ExternalInput")   # padded
    b2_d = nc.dram_tensor("b2", [1, H], f32, kind="ExternalInput")   # padded
    idx_d = nc.dram_tensor("idx", [2, cfg.NSB, P, NIDX // 16], mybir.dt.int16,
                           kind="ExternalInput")
    sel_d = nc.dram_tensor("sel", [2, cfg.NSB, P, NPT * P], f16,
                           kind="ExternalInput")
    out_d = nc.dram_tensor("out", [cfg.NSH, C], f32, kind="ExternalOutput")

    ag_in = [nc.dram_tensor(f"ag_in{l}", [cfg.HALF, 2 * F], f16)
             for l in range(NLAY)]
    tables = [nc.dram_tensor(f"table{l}", [cfg.TBL, 2 * F], f16,
                             addr_space="Shared")
              for l in range(NLAY)]

    with tile.TileContext(nc) as tc:
        import contextlib
        with contextlib.ExitStack() as ctx:
            const = ctx.enter_context(tc.tile_pool(name="const", bufs=1))
            htp = ctx.enter_context(tc.tile_pool(name="ht", bufs=1))
            psum = ctx.enter_context(tc.tile_pool(name="psum", bufs=8, space="PSUM"))
            meta = ctx.enter_context(tc.tile_pool(name="meta", bufs=4))
            gpool = ctx.enter_context(tc.tile_pool(name="g", bufs=5))
            selp = ctx.enter_context(tc.tile_pool(name="sel", bufs=4))
            yp = ctx.enter_context(tc.tile_pool(name="y", bufs=4))

            nc.gpsimd.load_library(library_config.mlp)

            # ---- constants
            nkt = DIN // P                       # k-tiles for layer-1 dense
            w1_sb = [const.tile([P, H], f16, name=f"w1sb{k}")
                     for k in range(nkt)]
            for k in range(nkt):
                nc.sync.dma_start(w1_sb[k][:], w1_d[k * P:(k + 1) * P, :])
            b1_sb = const.tile([1, H], f16)
            nc.sync.dma_start(b1_sb[:], b1_d[:])
            wm_sb = [const.tile([P, H], f32, name=f"wmsb{i}")
                     for i in range(L)]
            bm_sb = [const.tile([1, H], f32, name=f"bmsb{i}")
                     for i in range(L)]
            for i in range(L):
                nc.sync.dma_start(wm_sb[i][:], wm_d[i])
                nc.sync.dma_start(bm_sb[i][:], bm_d[i])
            w2_sb = const.tile([P, H], f32)
            nc.sync.dma_start(w2_sb[:], w2_d[:])
            b2_sb = const.tile([1, H], f32)
            nc.sync.dma_start(b2_sb[:], b2_d[:])
            ones_sb = const.tile([1, P], f32)
            nc.vector.memset(ones_sb[:], 1.0)
            ones16_sb = const.tile([1, P], f16)
            nc.vector.memset(ones16_sb[:], 1.0)


            ht = htp.tile([P, cfg.RPAD], f32)

            def dense_block(m, lhs_tiles, rhs_sb, bias_sb, l, ones=None):
                """y[m-block] = lhsT.T @ rhs + bias -> ag_in[l][h] rows."""
                ps = psum.tile([P, F], f32, tag="ps")
                for k, lt in enumerate(lhs_tiles):
                    nc.tensor.matmul(
                        out=ps[:], lhsT=lt, rhs=rhs_sb[k][:],
                        start=(k == 0), stop=False)
                nc.tensor.matmul(out=ps[:],
                                 lhsT=(ones if ones is not None else ones_sb)[:],
                                 rhs=bias_sb[:], start=False, stop=True)
                ysb = yp.tile([P, F], f16, tag="ysb")
                nc.vector.tensor_copy(out=ysb[:], in_=ps[:])
                r0 = m * P
                r1 = min(cfg.NSH, r0 + P)
                for h in (0, 1):
                    lo = max(r0, h * cfg.HALF)
                    hi = min(r1, (h + 1) * cfg.HALF)
                    if lo < hi:
                        nc.sync.dma_start(
                            out=ag_in[l][lo - h * cfg.HALF:hi - h * cfg.HALF,
                                         h * F:(h + 1) * F],
                            in_=ysb[lo - r0:hi - r0, :])

            def ht_dense_block(m, l):
                """Middle/final dense for layer l (reads ht)."""
                if l <= L:
                    dense_block(m, [ht[:, m * P:(m + 1) * P]],
                                [wm_sb[l - 1]], bm_sb[l - 1], l)
                else:
                    dense_block(m, [ht[:, m * P:(m + 1) * P]],
                                [w2_sb], b2_sb, l)

            def allgather(l):
                if no_cc:
                    nc.sync.dma_start(out=tables[l][0:cfg.HALF, :],
                                      in_=ag_in[l][:])
                    return
                nc.gpsimd.collective_compute(
                    "AllGather", mybir.AluOpType.bypass,
                    replica_groups=[list(range(NCORES))],
                    ins=[ag_in[l][:]], outs=[tables[l][:]])

            qrr = [0]

            def spmm(l):
                """tables[l] -> block outputs; updates ht (l<=L) or out (final).

                Interleaves next layer's dense per superblock so the next
                AllGather can launch right at the end of this spmm.
                """
                final = l == L + 1
                for s in range(cfg.NSB):
                    g = []
                    sl = []
                    for h in (0, 1):
                        it = meta.tile([P, NIDX // 16], mybir.dt.int16,
                                       tag="it")
                        nc.sync.dma_start(it[:], idx_d[h, s])
                        sel_sb = selp.tile([P, NPT, P], f16, tag="sel")
                        nc.scalar.dma_start(sel_sb[:], sel_d[h, s])
                        gt = gpool.tile([P, NPT, F], f16, tag="g")
                        # chunk calls to <=56 descs/lane: single_packet=True
                        # coalesces each lane's stream into ONE packet and the
                        # HW packet ceiling is 64 descriptors.  Round-robin
                        # the 4 SWDGE queues so descriptor prep spreads over
                        # all 8 Q7 cores (queue q preps on cores (2q, 2q+1)).
                        CH = 7
                        for c0 in range(0, NPT, CH):
                            c1 = min(NPT, c0 + CH)
                            nc.gpsimd.dma_gather(
                                gt[:, c0:c1, :],
                                tables[l][:, h * F:(h + 1) * F],
                                it[:, c0 * 8:c1 * 8],
                                (c1 - c0) * P, (c1 - c0) * P, F,
                                elem_step=2 * F, single_packet=True,
                                queue_num=qrr[0] % 4)
                            qrr[0] += 1
                        g.append(gt)
                        sl.append(sel_sb)
                    for bb in range(cfg.SBB):
                        b = s * cfg.SBB + bb
                        if final:
                            ps = psum.tile([P, F], f32, tag="ps")
                        else:
                            ps = psum.tile([F, P], f32, tag="ps")
                        k = 0
                        for h in (0, 1):
                            for t in range(TPB):
                                j = bb * TPB + t
                                sel = sl[h][:, j, :]
                                if final:
                                    nc.tensor.matmul(
                                        out=ps[:], lhsT=sel, rhs=g[h][:, j, :],
                                        start=(k == 0), stop=(k == 2 * TPB - 1))
                                else:
                                    nc.tensor.matmul(
                                        out=ps[:], lhsT=g[h][:, j, :], rhs=sel,
                                        start=(k == 0), stop=(k == 2 * TPB - 1))
                                k += 1
                        if final:
                            osb = yp.tile([P, F], f32, tag="osb")
                            nc.vector.tensor_copy(out=osb[:], in_=ps[:])
                            r0 = b * P
                            r1 = min(cfg.NSH, r0 + P)
                            if r0 < r1:
                                nc.sync.dma_start(out=out_d[r0:r1, :],
                                                  in_=osb[: r1 - r0, :C])
                        elif l == 0:
                            nc.vector.tensor_scalar(
                                out=ht[:, b * P:(b + 1) * P], in0=ps[:],
                                scalar1=0.0, scalar2=None,
                                op0=mybir.AluOpType.max)
                        else:
                            tmp = yp.tile([P, P], f32, tag="tmp")
                            nc.vector.tensor_scalar(
                                out=tmp[:], in0=ps[:],
                                scalar1=0.0, scalar2=dt_val,
                                op0=mybir.AluOpType.max,
                                op1=mybir.AluOpType.mult)
                            nc.vector.tensor_add(
                                out=ht[:, b * P:(b + 1) * P],
                                in0=ht[:, b * P:(b + 1) * P], in1=tmp[:])
                    # interleave next layer's dense so its AllGather can
                    # launch right after this spmm's last superblock
                    if not final:
                        for bb in range(cfg.SBB):
                            ht_dense_block(s * cfg.SBB + bb, l + 1)
                if not final:
                    allgather(l + 1)

            # ---- layer-0 dense (bulk fp16 xT load, scoped so SBUF frees)
            with tc.tile_pool(name="xt", bufs=1) as xtp:
                xt_sb = xtp.tile([P, nkt * cfg.RPAD], f16)
                for k in range(nkt):
                    nc.sync.dma_start(
                        xt_sb[:, k * cfg.RPAD:(k + 1) * cfg.RPAD],
                        xT_d[k * P:(k + 1) * P, :])
                for m in range(cfg.NBLK):
                    dense_block(
                        m,
                        [xt_sb[:, k * cfg.RPAD + m * P:k * cfg.RPAD + (m + 1) * P]
                         for k in range(nkt)],
                        w1_sb, b1_sb, 0, ones=ones16_sb)
            allgather(0)
            for l in range(NLAY):
                spmm(l)

    nc.compile()
    return nc


# ------------------------------------------------------------------ driver

_CACHE = {}


def _get_program(cfg, TPB, dt_val):
    key = (cfg.N, cfg.E, TPB, float(dt_val))
    if key not in _CACHE:
        _CACHE[key] = build_program(cfg, TPB, dt_val)
    return _CACHE[key]


def prepare(cfg, inputs):
    """Preprocess inputs and build (cached) program. Returns (nc, in_maps)."""
    x = np.asarray(inputs["x"], np.float32)
    per_core, TPB = preprocess(cfg, x, inputs["edge_row"], inputs["edge_col"],
                               inputs["edge_val"])
    dt_val = float(np.asarray(inputs["time_step"]))
    nc = _get_program(cfg, TPB, dt_val)

    w2 = np.zeros((cfg.H, cfg.H), np.float32)
    w2[:, : cfg.C] = np.asarray(inputs["w2"], np.float32)
    b2 = np.zeros((1, cfg.H), np.float32)
    b2[0, : cfg.C] = np.asarray(inputs["b2"], np.float32).reshape(-1)
    shared = dict(
        w1=np.asarray(inputs["w1"], np.float16),
        b1=np.asarray(inputs["b1"], np.float16).reshape(1, cfg.H),
        wm=np.asarray(inputs["wm"], np.float32),
        bm=np.asarray(inputs["bm"], np.float32).reshape(cfg.L, 1, cfg.H),
        w2=w2,
        b2=b2,
    )
    in_maps = [{**shared, **pc} for pc in per_core]
    return nc, in_maps


def run(cfg, inputs):
    nc, in_maps = prepare(cfg, inputs)
    res = run_bass_kernel_spmd(nc, in_maps, list(range(NCORES)))
    out = np.concatenate([res.results[r]["out"] for r in range(NCORES)], axis=0)
    return out.astype(np.float32)


def kernel(**inputs) -> np.ndarray:
    return run(CFG_FULL, inputs)


# ---------------------------------------------------- timing helper (test use)


def make_timed_runner(nc, in_maps):
    """Build a reusable jitted runner (no donation, device-resident operands).

    Mirrors bass2jax.run_bass_via_pjrt's multi-core path but keeps the jitted
    callable and device arrays so repeated calls measure dispatch+exec only.
    Returns (call_fn, out_unpack_fn).
    """
    import jax
    from jax.sharding import Mesh, PartitionSpec
    from jax.experimental.shard_map import shard_map
    from concourse import bass2jax
    from concourse.bass2jax import _bass_exec_p, partition_id_tensor

    bass2jax.install_neuronx_cc_hook()
    n_cores = len(in_maps)
    partition_name = nc.partition_id_tensor.name if nc.partition_id_tensor else None
    in_names, out_names, out_avals, zero_outs = [], [], [], []
    for alloc in nc.m.functions[0].allocations:
        if not isinstance(alloc, mybir.MemoryLocationSet):
            continue
        name = alloc.memorylocations[0].name
        if alloc.kind == "ExternalInput":
            if name != partition_name:
                in_names.append(name)
        elif alloc.kind == "ExternalOutput":
            out_names.append(name)
            out_avals.append(jax.core.ShapedArray(
                tuple(alloc.tensor_shape), mybir.dt.np(alloc.dtype)))
            zero_outs.append(np.zeros(tuple(alloc.tensor_shape),
                                      mybir.dt.np(alloc.dtype)))
    n_params = len(in_names)
    all_names = in_names + out_names
    if partition_name is not None:
        all_names.append(partition_name)

    def _body(*args):
        operands = list(args)
        if partition_name is not None:
            operands.append(partition_id_tensor())
        return tuple(_bass_exec_p.bind(
            *operands,
            out_avals=tuple(out_avals),
            in_names=tuple(all_names),
            out_names=tuple(out_names),
            lowering_input_output_aliases=(),
            sim_require_finite=True,
            sim_require_nnan=True,
            nc=nc,
        ))

    devices = jax.devices()[:n_cores]
    mesh = Mesh(np.asarray(devices), ("core",))
    spec_in = (PartitionSpec("core"),) * (n_params + len(out_names))
    spec_out = (PartitionSpec("core"),) * len(out_names)
    fn = jax.jit(shard_map(_body, mesh=mesh, in_specs=spec_in,
                           out_specs=spec_out, check_rep=False),
                 keep_unused=True)

    sharding = jax.sharding.NamedSharding(mesh, PartitionSpec("core"))
    dev_args = []
    for i, name in enumerate(in_names):
        cat = np.concatenate([np.asarray(m[name]) for m in in_maps], axis=0)
        dev_args.append(jax.device_put(cat, sharding))
    for z in zero_outs:
        cat = np.zeros((n_cores * z.shape[0], *z.shape[1:]), z.dtype)
        dev_args.append(jax.device_put(cat, sharding))

    def call():
        outs = fn(*dev_args)
        jax.block_until_ready(outs)
        return outs

    def unpack(outs):
        return [
            {name: np.asarray(outs[i]).reshape(n_cores, *out_avals[i].shape)[c]
             for i, name in enumerate(out_names)}
            for c in range(n_cores)
        ]

    return call, unpack
